# revision 27
# baseline (speedup 1.0000x reference)
"""AirTNN Trainium2 kernel (8 NeuronCores, SPMD + AllGather), fp8 edition.

Computation (reference): 3 sequential "shifts", each
    x_up <- (upper_lp * fad_k) @ x_up + noise_k
    x_low <- (lower_lp * fad_k) @ x_low + noise_k   (same noise)
with fad_k ~ Rayleigh from a fixed PRNG key and noise_k derived from the
running batch-0 signal power.  Output accumulates per-shift projections
x_up @ up_W[k].T + x_low @ low_W[k].T plus x @ h_W.T.

Numerics (validated against the reference in fp64 simulation, 2.7e-3):
 - The chain explodes along the all-ones direction (A >= 0 with large mean),
   so the output is dominated by a coherent component; incoherent rounding
   noise injected mid-chain is strongly suppressed.  fp8 A matrices and fp8
   gathered intermediates are safe; the *input* x0 is the one tensor whose
   quantization creates a persistent coherent error, so it is carried as an
   e4m3 hi+lo pair (residual 7.5e-4) that the DoubleRow datapath sums for
   free.  The last boundary is the error-sensitive one -> e3m4 there.
 - Per-shift operand dtypes: shift 0: A e4m3 paired against (x0_hi, x0_lo)
   via a 0-stride broadcast rhs (DoubleRow, 16 matmuls); shift 1: A e4m3 x
   gathered e4m3 (DoubleRow over chunk pairs); shift 2: A e3m4 x gathered
   e3m4 at 1x rate (32 matmuls).

Schedule per core (row-sharding, tensor-parallel 1D SpMM):
 - All A DMAs (>=1 MiB each, pre-tiled host-side) issue up-front on the
   sync (SP/HWDGE) queue and stream continuously; SBUF holds the full fp8
   A stream.
 - y = bt*psum + nz in one DVE op -> fp16 yt (kept for projections);
   PE-transpose of yt + ACT copy/cast assembles the fp8 gather payload.
 - cc_in writes and gathered readbacks ride the ACT (scalar/HWDGE) queue;
   AllGather triggers sit on gpsimd.  fp8 payloads keep the AllGather under
   the 1 MiB mesh-algorithm crossover (~9 us vs ~20 us RDH); per boundary
   up's AllGather overlaps low's matmuls.
 - Projections (wc blockdiag, scale-folded) run at the end from the six
   fp16 yt tiles plus the h_W term; host unscales by the global factor G.
"""

import os
import sys

import numpy as np

sys.path.insert(0, "/opt/trn_rl_repo")

NCORES = 8
N = 4096
C = 64
B = 2
K = 2                  # taps; K+1 shifts
NSHIFT = K + 1
R = N // NCORES        # 512 rows per core
C2 = C * B             # 128 (both batches side by side)
NJ = N // 128          # 32 contraction chunks
NTERM = 2 * NSHIFT + 1 # projection terms
SNR_LIN = 10.0
CF_COMP_STD = 0.5
SX = (16.0, 2.0, 1.0)  # per-shift yt scale (gather fp8 ranges; last unused)
SA = (32.0, 32.0, 4.0) # per-shift A fp8 scale (e4m3, e4m3, e3m4)

_compiled = {}
LAST_RESULTS = None    # BassKernelResults of the most recent device run


def _build_nc():
    import concourse.bacc as bacc
    import concourse.mybir as mybir
    import concourse.tile as tile

    fp16 = mybir.dt.float16
    fp32 = mybir.dt.float32
    fp8e3 = mybir.dt.float8e3
    fp8e4 = mybir.dt.float8e4
    DR = mybir.MatmulPerfMode.DoubleRow

    nc = bacc.Bacc("TRN2", target_bir_lowering=False, debug=False,
                   num_devices=NCORES)

    # pre-tiled A streams: row block br*128+p, col j*512+m, per-shift dtype
    a0 = nc.dram_tensor("a0", [2 * 128, NJ * R], fp8e4, kind="ExternalInput")
    a1 = nc.dram_tensor("a1", [2 * 128, NJ * R], fp8e4, kind="ExternalInput")
    a2 = nc.dram_tensor("a2", [2 * 128, NJ * R], fp8e3, kind="ExternalInput")
    # x0 as e4m3 (hi, lo) pairs per chunk: [p, j*(2*C2) + t*C2 + c2]
    x0 = nc.dram_tensor("x0", [128, NJ * 2 * C2], fp8e4, kind="ExternalInput")
    xt0 = nc.dram_tensor("xt0", [C2, R], fp16, kind="ExternalInput")
    nz = nc.dram_tensor("nz", [C2, NSHIFT * R], fp16, kind="ExternalInput")
    wc = nc.dram_tensor("wc", [C2, NTERM * C2], fp16, kind="ExternalInput")
    bt = nc.dram_tensor("bt", [128, NSHIFT], fp32, kind="ExternalInput")
    idn = nc.dram_tensor("idn", [128, 128], fp16, kind="ExternalInput")
    out_t = nc.dram_tensor("out_t", [C2, R], fp32, kind="ExternalOutput")

    # one collective per (boundary, branch): rank block [p, s*128+c2]
    ccdt = [fp8e4, fp8e3]
    cc_in = [[nc.dram_tensor(f"cc_in{k}{br}", [128, 4 * C2], ccdt[k])
              for br in range(2)] for k in range(NSHIFT - 1)]
    cc_out = [[nc.dram_tensor(f"cc_out{k}{br}", [NCORES * 128, 4 * C2],
                              ccdt[k], addr_space="Shared")
               for br in range(2)] for k in range(NSHIFT - 1)]

    with tile.TileContext(nc) as tc:
        with (
            tc.tile_pool(name="const", bufs=1) as constp,
            tc.tile_pool(name="apool", bufs=2 * NSHIFT * 2) as apool,
            tc.tile_pool(name="xgpool", bufs=4) as xgpool,
            tc.tile_pool(name="ccsb", bufs=2) as ccsbp,
            tc.tile_pool(name="psum", bufs=2, space="PSUM") as psump,
            tc.tile_pool(name="psumt", bufs=2, space="PSUM") as psumtp,
            tc.tile_pool(name="psumo", bufs=1, space="PSUM") as psumop,
        ):
            # ---- prologue: all bulk DMAs issued up-front on the SP queue,
            # ordered by first use so the stream never blocks a consumer.
            X0 = constp.tile([128, NJ, 2, C2], fp8e4, tag="x0")
            nc.sync.dma_start(
                X0[:, :NJ // 2, :, :],
                x0[:, :NJ * C2].rearrange("p (j t c) -> p j t c",
                                          j=NJ // 2, t=2, c=C2))

            at = {}

            def load_a(k, br, half):
                src = (a0, a1, a2)[k]
                row0 = br * 128
                col0 = half * (NJ // 2) * R
                sl = src[row0:row0 + 128, col0:col0 + (NJ // 2) * R]
                if k == 1:
                    t = apool.tile([128, NJ // 4, 2, R], fp8e4)
                    nc.sync.dma_start(
                        t[:], sl.rearrange("p (j two r) -> p j two r",
                                           j=NJ // 4, two=2, r=R))
                else:
                    t = apool.tile([128, NJ // 2, R], (fp8e4, fp8e4, fp8e3)[k])
                    nc.sync.dma_start(
                        t[:], sl.rearrange("p (j r) -> p j r",
                                           j=NJ // 2, r=R))
                at[(k, br, half)] = t

            load_a(0, 0, 0)
            nc.sync.dma_start(
                X0[:, NJ // 2:, :, :],
                x0[:, NJ * C2:].rearrange("p (j t c) -> p j t c",
                                          j=NJ // 2, t=2, c=C2))
            load_a(0, 0, 1)
            load_a(0, 1, 0)
            load_a(0, 1, 1)
            IDN = constp.tile([128, 128], fp16, tag="idn")
            nc.sync.dma_start(IDN[:], idn[:])
            NZ = constp.tile([C2, NSHIFT, R], fp16, tag="nz")
            nc.sync.dma_start(NZ[:], nz[:].rearrange("p (k r) -> p k r",
                                                     k=NSHIFT, r=R))
            BT = constp.tile([128, NSHIFT], fp32, tag="bt")
            nc.sync.dma_start(BT[:], bt[:])
            load_a(1, 0, 0)
            load_a(1, 0, 1)
            load_a(1, 1, 0)
            load_a(1, 1, 1)
            WC = constp.tile([C2, NTERM, C2], fp16, tag="wc")
            nc.sync.dma_start(WC[:], wc[:].rearrange("p (t c) -> p t c",
                                                     t=NTERM, c=C2))
            XT0 = constp.tile([C2, R], fp16, tag="xt0")
            nc.sync.dma_start(XT0[:], xt0[:])
            load_a(2, 0, 0)
            load_a(2, 0, 1)
            load_a(2, 1, 0)
            load_a(2, 1, 1)

            # ---- shift chain
            y16 = {}
            xgt = {}
            for k in range(NSHIFT):
                for br in range(2):
                    ps = psump.tile([C2, R], fp32)
                    if k == 0:
                        # DoubleRow: (x0_hi, x0_lo) against the same A chunk
                        # (0-stride broadcast) -> psum += A @ (hi + lo)
                        for j in range(NJ):
                            half = at[(0, br, j // (NJ // 2))]
                            rhs = half[:, j % (NJ // 2), :].unsqueeze(1) \
                                .broadcast_to([128, 2, R])
                            nc.tensor.matmul(ps[:], X0[:, j, :, :], rhs,
                                             start=(j == 0),
                                             stop=(j == NJ - 1),
                                             perf_mode=DR)
                    elif k == 1:
                        # DoubleRow over adjacent chunk pairs
                        for jj in range(NJ // 2):
                            half = at[(1, br, jj // (NJ // 4))]
                            rhs = half[:, jj % (NJ // 4), :, :]
                            lhsT = xgt[(1, br)][:, jj // 2, jj % 2, :, :]
                            nc.tensor.matmul(ps[:], lhsT, rhs,
                                             start=(jj == 0),
                                             stop=(jj == NJ // 2 - 1),
                                             perf_mode=DR)
                    else:
                        for j in range(NJ):
                            half = at[(2, br, j // (NJ // 2))]
                            rhs = half[:, j % (NJ // 2), :]
                            lhsT = xgt[(2, br)][:, j // 4, j % 4, :]
                            nc.tensor.matmul(ps[:], lhsT, rhs,
                                             start=(j == 0),
                                             stop=(j == NJ - 1))
                    # yt = bt_k * psum + nz_k  (fp16; the fp8 gather source)
                    yt = constp.tile([C2, R], fp16, tag=f"y{k}{br}")
                    nc.vector.scalar_tensor_tensor(
                        yt[:], ps[:], BT[:, k:k + 1], NZ[:, k, :],
                        op0=mybir.AluOpType.mult, op1=mybir.AluOpType.add)
                    y16[(k, br)] = yt
                    if k < NSHIFT - 1:
                        # transpose to natural layout, cast to fp8, gather
                        ccsb = ccsbp.tile([128, 4 * C2], ccdt[k])
                        for s in range(4):
                            pt = psumtp.tile([128, 128], fp16)
                            nc.tensor.transpose(
                                pt[:], yt[:, s * 128:(s + 1) * 128], IDN[:])
                            nc.scalar.activation(
                                ccsb[:, s * C2:(s + 1) * C2], pt[:],
                                mybir.ActivationFunctionType.Copy)
                        nc.scalar.dma_start(cc_in[k][br][:], ccsb[:])
                        nc.gpsimd.collective_compute(
                            "AllGather", mybir.AluOpType.bypass,
                            replica_groups=[list(range(NCORES))],
                            ins=[cc_in[k][br][:]], outs=[cc_out[k][br][:]])
                        # readback directly after this branch's AG, split in
                        # rank-halves: the next shift's first matmuls unblock
                        # as soon as the first half lands, instead of
                        # queueing behind the other branch's payload assembly
                        shp = ([128, NCORES, 2, 2, C2] if k == 0
                               else [128, NCORES, 4, C2])
                        tg = xgpool.tile(shp, ccdt[k])
                        src = cc_out[k][br][:].rearrange(
                            "(r p) m -> p r m", r=NCORES, p=128)
                        if k == 0:
                            src = src.rearrange(
                                "p r (a b c) -> p r a b c", a=2, b=2, c=C2)
                        else:
                            src = src.rearrange(
                                "p r (s c) -> p r s c", s=4, c=C2)
                        hN = NCORES // 2
                        nc.scalar.dma_start(tg[:, :hN], src[:, :hN])
                        nc.scalar.dma_start(tg[:, hN:], src[:, hN:])
                        xgt[(k + 1, br)] = tg

            # ---- projections (blockdiag, scale-folded) + h term
            po = psumop.tile([C2, R], fp32, tag="po")
            for k in range(NSHIFT):
                for br in range(2):
                    nc.tensor.matmul(po[:], WC[:, 2 * k + br, :],
                                     y16[(k, br)][:],
                                     start=(k == 0 and br == 0), stop=False)
            nc.tensor.matmul(po[:], WC[:, NTERM - 1, :], XT0[:],
                             start=False, stop=True)
            OT = constp.tile([C2, R], fp32, tag="ot")
            nc.vector.tensor_copy(OT[:], po[:])
            nc.sync.dma_start(out_t[:], OT[:])

    nc.compile()
    return nc


def _host_precompute(x, lower_lp, upper_lp, up_W, low_W, h_W):
    """PRNG reproduction + scaling; returns per-core input maps and G."""
    import jax
    import jax.numpy as jnp
    import ml_dtypes

    E3 = ml_dtypes.float8_e3m4
    E4 = ml_dtypes.float8_e4m3
    ADT = (E4, E4, E3)
    cpu = jax.devices("cpu")[0]
    f32 = np.float32

    with jax.default_device(cpu):
        key = jax.random.key(1)
        keys = jax.random.split(key, NSHIFT)
        fads, gs = [], []
        for i in range(NSHIFT):
            kf, kn = jax.random.split(keys[i])
            kr, ki = jax.random.split(kf)
            re = jax.random.normal(kr, (N, N), jnp.float32) * CF_COMP_STD
            im = jax.random.normal(ki, (N, N), jnp.float32) * CF_COMP_STD
            fads.append(np.asarray(jnp.sqrt(re * re + im * im)))
            gs.append(np.asarray(jax.random.normal(kn, (N, C), jnp.float32)))

    # fp32 replica of the up-branch batch-0 chain -> noise stds and scales
    stds = []
    z = x[0].astype(f32)
    for i in range(NSHIFT):
        stds.append(f32(np.sqrt(np.mean(z * z) / SNR_LIN)))
        z = (upper_lp * fads[i]).astype(f32) @ z + stds[i] * gs[i]
    r_last = f32(np.sqrt(np.mean(z * z)))
    r = [f32(stds[i + 1] * np.sqrt(SNR_LIN)) for i in range(NSHIFT - 1)]
    r.append(r_last)
    r_in = f32(np.sqrt(np.mean(x[0].astype(f32) ** 2)))
    G = float(r[-1])

    # big shift matrices: (lp * fad).T * sa_k in per-shift fp8, column-sliced
    # per core and pre-tiled partition-major:
    #   a_k[br*128+p, j*512+m] = (AT*sa_k)[j*128+p, d*R+m]
    a_cores = [[np.empty((2 * 128, NJ * R), ADT[k]) for k in range(NSHIFT)]
               for _ in range(NCORES)]
    for k in range(NSHIFT):
        for br, lp in ((0, upper_lp), (1, lower_lp)):
            at8 = np.ascontiguousarray(
                (lp * fads[k]).T * f32(SA[k])).astype(ADT[k])
            row0 = br * 128
            for d in range(NCORES):
                blk = at8[:, d * R:(d + 1) * R]          # [N, R]
                a_cores[d][k][row0:row0 + 128, :] = (
                    blk.reshape(NJ, 128, R).transpose(1, 0, 2)
                       .reshape(128, NJ * R))

    # normalized input, both batches side by side: X[n, c2]
    X = np.empty((N, C2), f32)
    X[:, :C] = x[0].astype(f32) / r_in
    X[:, C:] = x[1].astype(f32) / r_in
    # e4m3 hi/lo pair; SBUF layout [p, j*(2*C2) + t*C2 + c2]
    Xhi = X.astype(E4)
    Xlo = (X - Xhi.astype(f32)).astype(E4)
    x0p = np.empty((N, 2, C2), E4)
    x0p[:, 0, :] = Xhi
    x0p[:, 1, :] = Xlo
    x0_sb = np.ascontiguousarray(
        x0p.reshape(NJ, 128, 2 * C2).transpose(1, 0, 2).reshape(128, -1))

    # per-core transposed input slice for the h_W projection
    X16 = X.astype(np.float16)
    xt0_cores = [np.ascontiguousarray(X16[d * R:(d + 1) * R, :].T)
                 for d in range(NCORES)]

    # per-core noise slices (x SX_k), transposed + duplicated for batches
    nz_cores = [np.empty((C2, NSHIFT * R), np.float16) for _ in range(NCORES)]
    for k in range(NSHIFT):
        nT = np.ascontiguousarray(
            (f32(SX[k]) * (stds[k] / r[k]) * gs[k]).astype(f32).T)
        for d in range(NCORES):
            sl = nT[:, d * R:(d + 1) * R].astype(np.float16)
            nz_cores[d][:C, k * R:(k + 1) * R] = sl
            nz_cores[d][C:, k * R:(k + 1) * R] = sl

    # projection weights: shift terms fold 1/SX_k (yt = SX_k * y_norm)
    wc_np = np.zeros((C2, NTERM * C2), np.float16)
    terms = []
    for k in range(NSHIFT):
        terms.append((f32(r[k] / (G * SX[k])), up_W[k]))
        terms.append((f32(r[k] / (G * SX[k])), low_W[k]))
    terms.append((f32(r_in / G), h_W))
    for ti, (scale, W) in enumerate(terms):
        blk = (scale * W.astype(f32)).T.astype(np.float16)  # [c, o]
        wc_np[:C, ti * C2:ti * C2 + C] = blk
        wc_np[C:, ti * C2 + C:(ti + 1) * C2] = blk

    # per-shift psum scales:
    #   bt_0 = SX_0 * beta_0 / SA_0            (x0 carried unscaled)
    #   bt_k = (SX_k / SX_{k-1}) * beta_k / SA_k
    bt_np = np.empty((128, NSHIFT), f32)
    bt_np[:, 0] = f32(SX[0] * (r_in / r[0]) / SA[0])
    for k in range(1, NSHIFT):
        bt_np[:, k] = f32((SX[k] / SX[k - 1]) * (r[k - 1] / r[k]) / SA[k])

    in_maps = []
    for d in range(NCORES):
        in_maps.append({
            "a0": a_cores[d][0],
            "a1": a_cores[d][1],
            "a2": a_cores[d][2],
            "x0": x0_sb,
            "xt0": xt0_cores[d],
            "nz": nz_cores[d],
            "wc": wc_np,
            "bt": bt_np,
            "idn": np.eye(128, dtype=np.float16),
        })
    return in_maps, G


def kernel(x, lower_lp, upper_lp, up_W, low_W, h_W):
    global LAST_RESULTS
    from concourse.bass_utils import run_bass_kernel_spmd

    x = np.asarray(x, np.float32)
    lower_lp = np.asarray(lower_lp, np.float32)
    upper_lp = np.asarray(upper_lp, np.float32)
    up_W = np.asarray(up_W, np.float32)
    low_W = np.asarray(low_W, np.float32)
    h_W = np.asarray(h_W, np.float32)

    in_maps, G = _host_precompute(x, lower_lp, upper_lp, up_W, low_W, h_W)

    if "nc" not in _compiled:
        _compiled["nc"] = _build_nc()
    nc = _compiled["nc"]

    trace = os.environ.get("AIRTNN_TRACE", "0") == "1"
    res = run_bass_kernel_spmd(nc, in_maps, list(range(NCORES)), trace=trace)
    LAST_RESULTS = res

    # out[b, d*R + m, o] = G * out_t_d[o + 64*b, m]
    out = np.empty((B, N, C), np.float32)
    for d in range(NCORES):
        ot = res.results[d]["out_t"]  # [C2, R] fp32
        for b in range(B):
            out[b, d * R:(d + 1) * R, :] = (ot[b * C:(b + 1) * C, :].T) * G
    return out


# revision 28
# speedup vs baseline: 1.2807x; 1.2807x over previous
"""AirTNN Trainium2 kernel (8 NeuronCores, SPMD + AllGather), fp8 edition.

Computation (reference): 3 sequential "shifts", each
    x_up <- (upper_lp * fad_k) @ x_up + noise_k
    x_low <- (lower_lp * fad_k) @ x_low + noise_k   (same noise)
with fad_k ~ Rayleigh from a fixed PRNG key and noise_k derived from the
running batch-0 signal power.  Output accumulates per-shift projections
x_up @ up_W[k].T + x_low @ low_W[k].T plus x @ h_W.T.

Numerics (validated against the reference in fp64 simulation, 2.7e-3):
 - The chain explodes along the all-ones direction (A >= 0 with large mean),
   so the output is dominated by a coherent component; incoherent rounding
   noise injected mid-chain is strongly suppressed.  fp8 A matrices and fp8
   gathered intermediates are safe; the *input* x0 is the one tensor whose
   quantization creates a persistent coherent error, so it is carried as an
   e4m3 hi+lo pair (residual 7.5e-4) that the DoubleRow datapath sums for
   free.  The last boundary is the error-sensitive one -> e3m4 there.
 - Per-shift operand dtypes: shift 0: A e4m3 paired against (x0_hi, x0_lo)
   via a 0-stride broadcast rhs (DoubleRow, 16 matmuls); shift 1: A e4m3 x
   gathered e4m3 (DoubleRow over chunk pairs); shift 2: A e3m4 x gathered
   e3m4 at 1x rate (32 matmuls).

Schedule per core (row-sharding, tensor-parallel 1D SpMM):
 - All A DMAs (>=1 MiB each, pre-tiled host-side) issue up-front on the
   sync (SP/HWDGE) queue and stream continuously; SBUF holds the full fp8
   A stream.
 - y = bt*psum + nz in one DVE op -> fp16 yt (kept for projections);
   PE-transpose of yt + ACT copy/cast assembles the fp8 gather payload.
 - cc_in writes and gathered readbacks ride the ACT (scalar/HWDGE) queue;
   AllGather triggers sit on gpsimd.  fp8 payloads keep the AllGather under
   the 1 MiB mesh-algorithm crossover (~9 us vs ~20 us RDH); per boundary
   up's AllGather overlaps low's matmuls.
 - Projections (wc blockdiag, scale-folded) run at the end from the six
   fp16 yt tiles plus the h_W term; host unscales by the global factor G.
"""

import os
import sys

import numpy as np

sys.path.insert(0, "/opt/trn_rl_repo")

NCORES = 8
N = 4096
C = 64
B = 2
K = 2                  # taps; K+1 shifts
NSHIFT = K + 1
R = N // NCORES        # 512 rows per core
C2 = C * B             # 128 (both batches side by side)
NJ = N // 128          # 32 contraction chunks
NTERM = 2 * NSHIFT + 1 # projection terms
SNR_LIN = 10.0
CF_COMP_STD = 0.5
SX = (16.0, 2.0, 1.0)  # per-shift yt scale (gather fp8 ranges; last unused)
SA = (32.0, 32.0, 4.0) # per-shift A fp8 scale (e4m3, e4m3, e3m4)

_compiled = {}
LAST_RESULTS = None    # BassKernelResults of the most recent device run


def _build_nc():
    import concourse.bacc as bacc
    import concourse.mybir as mybir
    import concourse.tile as tile

    fp16 = mybir.dt.float16
    fp32 = mybir.dt.float32
    fp8e3 = mybir.dt.float8e3
    fp8e4 = mybir.dt.float8e4
    DR = mybir.MatmulPerfMode.DoubleRow

    nc = bacc.Bacc("TRN2", target_bir_lowering=False, debug=False,
                   num_devices=NCORES)

    # pre-tiled A streams: row block br*128+p, col j*512+m, per-shift dtype
    a0 = nc.dram_tensor("a0", [2 * 128, NJ * R], fp8e4, kind="ExternalInput")
    a1 = nc.dram_tensor("a1", [2 * 128, NJ * R], fp8e4, kind="ExternalInput")
    a2 = nc.dram_tensor("a2", [2 * 128, NJ * R], fp8e3, kind="ExternalInput")
    # x0 as e4m3 (hi, lo) pairs per chunk: [p, j*(2*C2) + t*C2 + c2]
    x0 = nc.dram_tensor("x0", [128, NJ * 2 * C2], fp8e4, kind="ExternalInput")
    xt0 = nc.dram_tensor("xt0", [C2, R], fp16, kind="ExternalInput")
    nz = nc.dram_tensor("nz", [C2, NSHIFT * R], fp16, kind="ExternalInput")
    wc = nc.dram_tensor("wc", [C2, NTERM * C2], fp16, kind="ExternalInput")
    bt = nc.dram_tensor("bt", [128, NSHIFT], fp32, kind="ExternalInput")
    idn = nc.dram_tensor("idn", [128, 128], fp16, kind="ExternalInput")
    out_t = nc.dram_tensor("out_t", [C2, R], fp32, kind="ExternalOutput")

    # one collective per (boundary, branch): rank block [p, s*128+c2]
    ccdt = [fp8e4, fp8e3]
    cc_in = [[nc.dram_tensor(f"cc_in{k}{br}", [128, 4 * C2], ccdt[k])
              for br in range(2)] for k in range(NSHIFT - 1)]
    cc_out = [[nc.dram_tensor(f"cc_out{k}{br}", [NCORES * 128, 4 * C2],
                              ccdt[k], addr_space="Shared")
               for br in range(2)] for k in range(NSHIFT - 1)]

    with tile.TileContext(nc) as tc:
        with (
            tc.tile_pool(name="const", bufs=1) as constp,
            tc.tile_pool(name="apool", bufs=2 * NSHIFT * 2) as apool,
            tc.tile_pool(name="xgpool", bufs=4) as xgpool,
            tc.tile_pool(name="ccsb", bufs=2) as ccsbp,
            tc.tile_pool(name="psum", bufs=2, space="PSUM") as psump,
            tc.tile_pool(name="psumt", bufs=2, space="PSUM") as psumtp,
            tc.tile_pool(name="psumo", bufs=1, space="PSUM") as psumop,
        ):
            # ---- prologue: all bulk DMAs issued up-front on the SP queue,
            # ordered by first use so the stream never blocks a consumer.
            X0 = constp.tile([128, NJ, 2, C2], fp8e4, tag="x0")
            nc.sync.dma_start(
                X0[:, :NJ // 2, :, :],
                x0[:, :NJ * C2].rearrange("p (j t c) -> p j t c",
                                          j=NJ // 2, t=2, c=C2))

            at = {}

            def load_a(k, br, half):
                src = (a0, a1, a2)[k]
                row0 = br * 128
                col0 = half * (NJ // 2) * R
                sl = src[row0:row0 + 128, col0:col0 + (NJ // 2) * R]
                if k == 1:
                    t = apool.tile([128, NJ // 4, 2, R], fp8e4)
                    nc.sync.dma_start(
                        t[:], sl.rearrange("p (j two r) -> p j two r",
                                           j=NJ // 4, two=2, r=R))
                else:
                    t = apool.tile([128, NJ // 2, R], (fp8e4, fp8e4, fp8e3)[k])
                    nc.sync.dma_start(
                        t[:], sl.rearrange("p (j r) -> p j r",
                                           j=NJ // 2, r=R))
                at[(k, br, half)] = t

            load_a(0, 0, 0)
            nc.sync.dma_start(
                X0[:, NJ // 2:, :, :],
                x0[:, NJ * C2:].rearrange("p (j t c) -> p j t c",
                                          j=NJ // 2, t=2, c=C2))
            load_a(0, 0, 1)
            load_a(0, 1, 0)
            load_a(0, 1, 1)
            IDN = constp.tile([128, 128], fp16, tag="idn")
            nc.sync.dma_start(IDN[:], idn[:])
            NZ = constp.tile([C2, NSHIFT, R], fp16, tag="nz")
            nc.sync.dma_start(NZ[:], nz[:].rearrange("p (k r) -> p k r",
                                                     k=NSHIFT, r=R))
            BT = constp.tile([128, NSHIFT], fp32, tag="bt")
            nc.sync.dma_start(BT[:], bt[:])
            load_a(1, 0, 0)
            load_a(1, 0, 1)
            load_a(1, 1, 0)
            load_a(1, 1, 1)
            WC = constp.tile([C2, NTERM, C2], fp16, tag="wc")
            nc.sync.dma_start(WC[:], wc[:].rearrange("p (t c) -> p t c",
                                                     t=NTERM, c=C2))
            XT0 = constp.tile([C2, R], fp16, tag="xt0")
            nc.sync.dma_start(XT0[:], xt0[:])
            load_a(2, 0, 0)
            load_a(2, 0, 1)
            load_a(2, 1, 0)
            load_a(2, 1, 1)

            # ---- shift chain
            y16 = {}
            xgt = {}
            for k in range(NSHIFT):
                for br in range(2):
                    ps = psump.tile([C2, R], fp32)
                    if k == 0:
                        # DoubleRow: (x0_hi, x0_lo) against the same A chunk
                        # (0-stride broadcast) -> psum += A @ (hi + lo)
                        for j in range(NJ):
                            half = at[(0, br, j // (NJ // 2))]
                            rhs = half[:, j % (NJ // 2), :].unsqueeze(1) \
                                .broadcast_to([128, 2, R])
                            nc.tensor.matmul(ps[:], X0[:, j, :, :], rhs,
                                             start=(j == 0),
                                             stop=(j == NJ - 1),
                                             perf_mode=DR)
                    elif k == 1:
                        # DoubleRow over adjacent chunk pairs
                        for jj in range(NJ // 2):
                            half = at[(1, br, jj // (NJ // 4))]
                            rhs = half[:, jj % (NJ // 4), :, :]
                            lhsT = xgt[(1, br)][:, jj // 2, jj % 2, :, :]
                            nc.tensor.matmul(ps[:], lhsT, rhs,
                                             start=(jj == 0),
                                             stop=(jj == NJ // 2 - 1),
                                             perf_mode=DR)
                    else:
                        for j in range(NJ):
                            half = at[(2, br, j // (NJ // 2))]
                            rhs = half[:, j % (NJ // 2), :]
                            lhsT = xgt[(2, br)][:, j // 4, j % 4, :]
                            nc.tensor.matmul(ps[:], lhsT, rhs,
                                             start=(j == 0),
                                             stop=(j == NJ - 1))
                    # yt = bt_k * psum + nz_k  (fp16; the fp8 gather source)
                    yt = constp.tile([C2, R], fp16, tag=f"y{k}{br}")
                    nc.vector.scalar_tensor_tensor(
                        yt[:], ps[:], BT[:, k:k + 1], NZ[:, k, :],
                        op0=mybir.AluOpType.mult, op1=mybir.AluOpType.add)
                    y16[(k, br)] = yt
                    if k < NSHIFT - 1:
                        # transpose to natural layout, cast to fp8, gather
                        ccsb = ccsbp.tile([128, 4 * C2], ccdt[k])
                        for s in range(4):
                            pt = psumtp.tile([128, 128], fp16)
                            nc.tensor.transpose(
                                pt[:], yt[:, s * 128:(s + 1) * 128], IDN[:])
                            nc.scalar.activation(
                                ccsb[:, s * C2:(s + 1) * C2], pt[:],
                                mybir.ActivationFunctionType.Copy)
                        nc.scalar.dma_start(cc_in[k][br][:], ccsb[:])
                        nc.gpsimd.collective_compute(
                            "AllGather", mybir.AluOpType.bypass,
                            replica_groups=[list(range(NCORES))],
                            ins=[cc_in[k][br][:]], outs=[cc_out[k][br][:]])
                        # readback directly after this branch's AG, split in
                        # rank-halves: the next shift's first matmuls unblock
                        # as soon as the first half lands, instead of
                        # queueing behind the other branch's payload assembly
                        shp = ([128, NCORES, 2, 2, C2] if k == 0
                               else [128, NCORES, 4, C2])
                        tg = xgpool.tile(shp, ccdt[k])
                        src = cc_out[k][br][:].rearrange(
                            "(r p) m -> p r m", r=NCORES, p=128)
                        if k == 0:
                            src = src.rearrange(
                                "p r (a b c) -> p r a b c", a=2, b=2, c=C2)
                        else:
                            src = src.rearrange(
                                "p r (s c) -> p r s c", s=4, c=C2)
                        nc.scalar.dma_start(tg[:], src)
                        xgt[(k + 1, br)] = tg

            # ---- projections (blockdiag, scale-folded) + h term
            po = psumop.tile([C2, R], fp32, tag="po")
            for k in range(NSHIFT):
                for br in range(2):
                    nc.tensor.matmul(po[:], WC[:, 2 * k + br, :],
                                     y16[(k, br)][:],
                                     start=(k == 0 and br == 0), stop=False)
            nc.tensor.matmul(po[:], WC[:, NTERM - 1, :], XT0[:],
                             start=False, stop=True)
            OT = constp.tile([C2, R], fp32, tag="ot")
            nc.vector.tensor_copy(OT[:], po[:])
            nc.sync.dma_start(out_t[:], OT[:])

    nc.compile()
    return nc


def _host_precompute(x, lower_lp, upper_lp, up_W, low_W, h_W):
    """PRNG reproduction + scaling; returns per-core input maps and G."""
    import jax
    import jax.numpy as jnp
    import ml_dtypes

    E3 = ml_dtypes.float8_e3m4
    E4 = ml_dtypes.float8_e4m3
    ADT = (E4, E4, E3)
    cpu = jax.devices("cpu")[0]
    f32 = np.float32

    with jax.default_device(cpu):
        key = jax.random.key(1)
        keys = jax.random.split(key, NSHIFT)
        fads, gs = [], []
        for i in range(NSHIFT):
            kf, kn = jax.random.split(keys[i])
            kr, ki = jax.random.split(kf)
            re = jax.random.normal(kr, (N, N), jnp.float32) * CF_COMP_STD
            im = jax.random.normal(ki, (N, N), jnp.float32) * CF_COMP_STD
            fads.append(np.asarray(jnp.sqrt(re * re + im * im)))
            gs.append(np.asarray(jax.random.normal(kn, (N, C), jnp.float32)))

    # fp32 replica of the up-branch batch-0 chain -> noise stds and scales
    stds = []
    z = x[0].astype(f32)
    for i in range(NSHIFT):
        stds.append(f32(np.sqrt(np.mean(z * z) / SNR_LIN)))
        z = (upper_lp * fads[i]).astype(f32) @ z + stds[i] * gs[i]
    r_last = f32(np.sqrt(np.mean(z * z)))
    r = [f32(stds[i + 1] * np.sqrt(SNR_LIN)) for i in range(NSHIFT - 1)]
    r.append(r_last)
    r_in = f32(np.sqrt(np.mean(x[0].astype(f32) ** 2)))
    G = float(r[-1])

    # big shift matrices: (lp * fad).T * sa_k in per-shift fp8, column-sliced
    # per core and pre-tiled partition-major:
    #   a_k[br*128+p, j*512+m] = (AT*sa_k)[j*128+p, d*R+m]
    a_cores = [[np.empty((2 * 128, NJ * R), ADT[k]) for k in range(NSHIFT)]
               for _ in range(NCORES)]
    for k in range(NSHIFT):
        for br, lp in ((0, upper_lp), (1, lower_lp)):
            at8 = np.ascontiguousarray(
                (lp * fads[k]).T * f32(SA[k])).astype(ADT[k])
            row0 = br * 128
            for d in range(NCORES):
                blk = at8[:, d * R:(d + 1) * R]          # [N, R]
                a_cores[d][k][row0:row0 + 128, :] = (
                    blk.reshape(NJ, 128, R).transpose(1, 0, 2)
                       .reshape(128, NJ * R))

    # normalized input, both batches side by side: X[n, c2]
    X = np.empty((N, C2), f32)
    X[:, :C] = x[0].astype(f32) / r_in
    X[:, C:] = x[1].astype(f32) / r_in
    # e4m3 hi/lo pair; SBUF layout [p, j*(2*C2) + t*C2 + c2]
    Xhi = X.astype(E4)
    Xlo = (X - Xhi.astype(f32)).astype(E4)
    x0p = np.empty((N, 2, C2), E4)
    x0p[:, 0, :] = Xhi
    x0p[:, 1, :] = Xlo
    x0_sb = np.ascontiguousarray(
        x0p.reshape(NJ, 128, 2 * C2).transpose(1, 0, 2).reshape(128, -1))

    # per-core transposed input slice for the h_W projection
    X16 = X.astype(np.float16)
    xt0_cores = [np.ascontiguousarray(X16[d * R:(d + 1) * R, :].T)
                 for d in range(NCORES)]

    # per-core noise slices (x SX_k), transposed + duplicated for batches
    nz_cores = [np.empty((C2, NSHIFT * R), np.float16) for _ in range(NCORES)]
    for k in range(NSHIFT):
        nT = np.ascontiguousarray(
            (f32(SX[k]) * (stds[k] / r[k]) * gs[k]).astype(f32).T)
        for d in range(NCORES):
            sl = nT[:, d * R:(d + 1) * R].astype(np.float16)
            nz_cores[d][:C, k * R:(k + 1) * R] = sl
            nz_cores[d][C:, k * R:(k + 1) * R] = sl

    # projection weights: shift terms fold 1/SX_k (yt = SX_k * y_norm)
    wc_np = np.zeros((C2, NTERM * C2), np.float16)
    terms = []
    for k in range(NSHIFT):
        terms.append((f32(r[k] / (G * SX[k])), up_W[k]))
        terms.append((f32(r[k] / (G * SX[k])), low_W[k]))
    terms.append((f32(r_in / G), h_W))
    for ti, (scale, W) in enumerate(terms):
        blk = (scale * W.astype(f32)).T.astype(np.float16)  # [c, o]
        wc_np[:C, ti * C2:ti * C2 + C] = blk
        wc_np[C:, ti * C2 + C:(ti + 1) * C2] = blk

    # per-shift psum scales:
    #   bt_0 = SX_0 * beta_0 / SA_0            (x0 carried unscaled)
    #   bt_k = (SX_k / SX_{k-1}) * beta_k / SA_k
    bt_np = np.empty((128, NSHIFT), f32)
    bt_np[:, 0] = f32(SX[0] * (r_in / r[0]) / SA[0])
    for k in range(1, NSHIFT):
        bt_np[:, k] = f32((SX[k] / SX[k - 1]) * (r[k - 1] / r[k]) / SA[k])

    in_maps = []
    for d in range(NCORES):
        in_maps.append({
            "a0": a_cores[d][0],
            "a1": a_cores[d][1],
            "a2": a_cores[d][2],
            "x0": x0_sb,
            "xt0": xt0_cores[d],
            "nz": nz_cores[d],
            "wc": wc_np,
            "bt": bt_np,
            "idn": np.eye(128, dtype=np.float16),
        })
    return in_maps, G


def kernel(x, lower_lp, upper_lp, up_W, low_W, h_W):
    global LAST_RESULTS
    from concourse.bass_utils import run_bass_kernel_spmd

    x = np.asarray(x, np.float32)
    lower_lp = np.asarray(lower_lp, np.float32)
    upper_lp = np.asarray(upper_lp, np.float32)
    up_W = np.asarray(up_W, np.float32)
    low_W = np.asarray(low_W, np.float32)
    h_W = np.asarray(h_W, np.float32)

    in_maps, G = _host_precompute(x, lower_lp, upper_lp, up_W, low_W, h_W)

    if "nc" not in _compiled:
        _compiled["nc"] = _build_nc()
    nc = _compiled["nc"]

    trace = os.environ.get("AIRTNN_TRACE", "0") == "1"
    res = run_bass_kernel_spmd(nc, in_maps, list(range(NCORES)), trace=trace)
    LAST_RESULTS = res

    # out[b, d*R + m, o] = G * out_t_d[o + 64*b, m]
    out = np.empty((B, N, C), np.float32)
    for d in range(NCORES):
        ot = res.results[d]["out_t"]  # [C2, R] fp32
        for b in range(B):
            out[b, d * R:(d + 1) * R, :] = (ot[b * C:(b + 1) * C, :].T) * G
    return out


# revision 35
# speedup vs baseline: 1.3391x; 1.0456x over previous
"""AirTNN Trainium2 kernel (8 NeuronCores, SPMD + AllGather), fp8 edition.

Computation (reference): 3 sequential "shifts", each
    x_up <- (upper_lp * fad_k) @ x_up + noise_k
    x_low <- (lower_lp * fad_k) @ x_low + noise_k   (same noise)
with fad_k ~ Rayleigh from a fixed PRNG key and noise_k derived from the
running batch-0 signal power.  Output accumulates per-shift projections
x_up @ up_W[k].T + x_low @ low_W[k].T plus x @ h_W.T.

Numerics (validated against the reference in fp64 simulation, 2.7e-3):
 - The chain explodes along the all-ones direction (A >= 0 with large mean),
   so the output is dominated by a coherent component; incoherent rounding
   noise injected mid-chain is strongly suppressed.  fp8 A matrices and fp8
   gathered intermediates are safe; the *input* x0 is the one tensor whose
   quantization creates a persistent coherent error, so it is carried as an
   e4m3 hi+lo pair (residual 7.5e-4) that the DoubleRow datapath sums for
   free.  The last boundary is the error-sensitive one -> e3m4 there.
 - Per-shift operand dtypes: shift 0: A e4m3 paired against (x0_hi, x0_lo)
   via a 0-stride broadcast rhs (DoubleRow, 16 matmuls); shift 1: A e4m3 x
   gathered e4m3 (DoubleRow over chunk pairs); shift 2: A e3m4 x gathered
   e3m4 at 1x rate (32 matmuls).

Schedule per core (row-sharding, tensor-parallel 1D SpMM):
 - All A DMAs (>=1 MiB each, pre-tiled host-side) issue up-front on the
   sync (SP/HWDGE) queue and stream continuously; SBUF holds the full fp8
   A stream.
 - y = bt*psum + nz in one DVE op -> fp16 yt (kept for projections);
   PE-transpose of yt + ACT copy/cast assembles the fp8 gather payload.
 - cc_in writes and gathered readbacks ride the ACT (scalar/HWDGE) queue;
   AllGather triggers sit on gpsimd.  fp8 payloads keep the AllGather under
   the 1 MiB mesh-algorithm crossover (~9 us vs ~20 us RDH); per boundary
   up's AllGather overlaps low's matmuls.
 - Projections (wc blockdiag, scale-folded) run at the end from the six
   fp16 yt tiles plus the h_W term; host unscales by the global factor G.
"""

import os
import sys

import numpy as np

sys.path.insert(0, "/opt/trn_rl_repo")

NCORES = 8
N = 4096
C = 64
B = 2
K = 2                  # taps; K+1 shifts
NSHIFT = K + 1
R = N // NCORES        # 512 rows per core
C2 = C * B             # 128 (both batches side by side)
NJ = N // 128          # 32 contraction chunks
NTERM = 2 * NSHIFT + 1 # projection terms
SNR_LIN = 10.0
CF_COMP_STD = 0.5
SX = (16.0, 16.0, 1.0)  # per-shift yt scale (gather fp8 ranges; last unused)
SA = (32.0, 32.0, 32.0) # per-shift A fp8 scale (all e4m3 for DoubleRow)

_compiled = {}
LAST_RESULTS = None    # BassKernelResults of the most recent device run


def _build_nc():
    import concourse.bacc as bacc
    import concourse.mybir as mybir
    import concourse.tile as tile

    fp16 = mybir.dt.float16
    fp32 = mybir.dt.float32
    fp8e3 = mybir.dt.float8e3
    fp8e4 = mybir.dt.float8e4
    DR = mybir.MatmulPerfMode.DoubleRow

    nc = bacc.Bacc("TRN2", target_bir_lowering=False, debug=False,
                   num_devices=NCORES)

    # pre-tiled A streams: row block br*128+p, col j*512+m, per-shift dtype
    a0 = nc.dram_tensor("a0", [2 * 128, NJ * R], fp8e4, kind="ExternalInput")
    a1 = nc.dram_tensor("a1", [2 * 128, NJ * R], fp8e4, kind="ExternalInput")
    a2 = nc.dram_tensor("a2", [2 * 128, NJ * R], fp8e4, kind="ExternalInput")
    # x0 as e4m3 (hi, lo) pairs per chunk: [p, j*(2*C2) + t*C2 + c2]
    x0 = nc.dram_tensor("x0", [128, NJ * 2 * C2], fp8e4, kind="ExternalInput")
    xt0 = nc.dram_tensor("xt0", [C2, R], fp16, kind="ExternalInput")
    nz = nc.dram_tensor("nz", [C2, NSHIFT * R], fp16, kind="ExternalInput")
    wc = nc.dram_tensor("wc", [C2, NTERM * C2], fp16, kind="ExternalInput")
    bt = nc.dram_tensor("bt", [128, NSHIFT], fp32, kind="ExternalInput")
    idn = nc.dram_tensor("idn", [128, 128], fp16, kind="ExternalInput")
    out_t = nc.dram_tensor("out_t", [C2, R], fp32, kind="ExternalOutput")

    # one collective per (boundary, branch): rank block [p, s*128+c2]
    ccdt = [fp8e4, fp8e4]
    cc_in = [[nc.dram_tensor(f"cc_in{k}{br}", [128, 4 * C2], ccdt[k])
              for br in range(2)] for k in range(NSHIFT - 1)]
    cc_out = [[nc.dram_tensor(f"cc_out{k}{br}", [NCORES * 128, 4 * C2],
                              ccdt[k], addr_space="Shared")
               for br in range(2)] for k in range(NSHIFT - 1)]

    with tile.TileContext(nc) as tc:
        with (
            tc.tile_pool(name="const", bufs=1) as constp,
            tc.tile_pool(name="apool", bufs=2 * NSHIFT * 2) as apool,
            tc.tile_pool(name="xgpool", bufs=4) as xgpool,
            tc.tile_pool(name="ccsb", bufs=2) as ccsbp,
            tc.tile_pool(name="psum", bufs=2, space="PSUM") as psump,
            tc.tile_pool(name="psumt", bufs=2, space="PSUM") as psumtp,
            tc.tile_pool(name="psumo", bufs=1, space="PSUM") as psumop,
        ):
            # ---- prologue: all bulk DMAs issued up-front on the SP queue,
            # ordered by first use so the stream never blocks a consumer.
            X0 = constp.tile([128, NJ, 2, C2], fp8e4, tag="x0")
            nc.sync.dma_start(
                X0[:, :NJ // 2, :, :],
                x0[:, :NJ * C2].rearrange("p (j t c) -> p j t c",
                                          j=NJ // 2, t=2, c=C2))

            at = {}

            def load_a(k, br, half):
                src = (a0, a1, a2)[k]
                row0 = br * 128
                col0 = half * (NJ // 2) * R
                sl = src[row0:row0 + 128, col0:col0 + (NJ // 2) * R]
                if k >= 1:
                    t = apool.tile([128, NJ // 4, 2, R], fp8e4)
                    nc.sync.dma_start(
                        t[:], sl.rearrange("p (j two r) -> p j two r",
                                           j=NJ // 4, two=2, r=R))
                else:
                    t = apool.tile([128, NJ // 2, R], fp8e4)
                    nc.sync.dma_start(
                        t[:], sl.rearrange("p (j r) -> p j r",
                                           j=NJ // 2, r=R))
                at[(k, br, half)] = t

            load_a(0, 0, 0)
            nc.sync.dma_start(
                X0[:, NJ // 2:, :, :],
                x0[:, NJ * C2:].rearrange("p (j t c) -> p j t c",
                                          j=NJ // 2, t=2, c=C2))
            load_a(0, 0, 1)
            load_a(0, 1, 0)
            load_a(0, 1, 1)
            IDN = constp.tile([128, 128], fp16, tag="idn")
            nc.sync.dma_start(IDN[:], idn[:])
            NZ = constp.tile([C2, NSHIFT, R], fp16, tag="nz")
            nc.sync.dma_start(NZ[:], nz[:].rearrange("p (k r) -> p k r",
                                                     k=NSHIFT, r=R))
            BT = constp.tile([128, NSHIFT], fp32, tag="bt")
            nc.sync.dma_start(BT[:], bt[:])
            load_a(1, 0, 0)
            load_a(1, 0, 1)
            load_a(1, 1, 0)
            load_a(1, 1, 1)
            WC = constp.tile([C2, NTERM, C2], fp16, tag="wc")
            nc.sync.dma_start(WC[:], wc[:].rearrange("p (t c) -> p t c",
                                                     t=NTERM, c=C2))
            XT0 = constp.tile([C2, R], fp16, tag="xt0")
            nc.sync.dma_start(XT0[:], xt0[:])
            load_a(2, 0, 0)
            load_a(2, 0, 1)
            load_a(2, 1, 0)
            load_a(2, 1, 1)

            # ---- shift chain
            y16 = {}
            xgt = {}
            for k in range(NSHIFT):
                for br in range(2):
                    ps = psump.tile([C2, R], fp32)
                    if k == 0:
                        # DoubleRow: (x0_hi, x0_lo) against the same A chunk
                        # (0-stride broadcast) -> psum += A @ (hi + lo)
                        for j in range(NJ):
                            half = at[(0, br, j // (NJ // 2))]
                            rhs = half[:, j % (NJ // 2), :].unsqueeze(1) \
                                .broadcast_to([128, 2, R])
                            nc.tensor.matmul(ps[:], X0[:, j, :, :], rhs,
                                             start=(j == 0),
                                             stop=(j == NJ - 1),
                                             perf_mode=DR)
                    else:
                        # DoubleRow over adjacent chunk pairs
                        for jj in range(NJ // 2):
                            half = at[(k, br, jj // (NJ // 4))]
                            rhs = half[:, jj % (NJ // 4), :, :]
                            lhsT = xgt[(k, br)][:, jj // 2, jj % 2, :, :]
                            nc.tensor.matmul(ps[:], lhsT, rhs,
                                             start=(jj == 0),
                                             stop=(jj == NJ // 2 - 1),
                                             perf_mode=DR)
                    # yt = bt_k * psum + nz_k  (fp16; the fp8 gather source)
                    yt = constp.tile([C2, R], fp16, tag=f"y{k}{br}")
                    nc.vector.scalar_tensor_tensor(
                        yt[:], ps[:], BT[:, k:k + 1], NZ[:, k, :],
                        op0=mybir.AluOpType.mult, op1=mybir.AluOpType.add)
                    y16[(k, br)] = yt
                    if k < NSHIFT - 1:
                        # transpose to natural layout, cast to fp8, gather
                        ccsb = ccsbp.tile([128, 4 * C2], ccdt[k])
                        for s in range(4):
                            pt = psumtp.tile([128, 128], fp16)
                            nc.tensor.transpose(
                                pt[:], yt[:, s * 128:(s + 1) * 128], IDN[:])
                            nc.scalar.activation(
                                ccsb[:, s * C2:(s + 1) * C2], pt[:],
                                mybir.ActivationFunctionType.Copy)
                        nc.scalar.dma_start(cc_in[k][br][:], ccsb[:])
                        nc.gpsimd.collective_compute(
                            "AllGather", mybir.AluOpType.bypass,
                            replica_groups=[list(range(NCORES))],
                            ins=[cc_in[k][br][:]], outs=[cc_out[k][br][:]])
                        # readback directly after this branch's AG, split in
                        # rank-halves: the next shift's first matmuls unblock
                        # as soon as the first half lands, instead of
                        # queueing behind the other branch's payload assembly
                        tg = xgpool.tile([128, NCORES, 2, 2, C2], ccdt[k])
                        src = cc_out[k][br][:].rearrange(
                            "(r p) m -> p r m", r=NCORES, p=128).rearrange(
                            "p r (a b c) -> p r a b c", a=2, b=2, c=C2)
                        nc.scalar.dma_start(tg[:], src)
                        xgt[(k + 1, br)] = tg

            # ---- projections (blockdiag, scale-folded) + h term
            po = psumop.tile([C2, R], fp32, tag="po")
            for k in range(NSHIFT):
                for br in range(2):
                    nc.tensor.matmul(po[:], WC[:, 2 * k + br, :],
                                     y16[(k, br)][:],
                                     start=(k == 0 and br == 0), stop=False)
            nc.tensor.matmul(po[:], WC[:, NTERM - 1, :], XT0[:],
                             start=False, stop=True)
            OT = constp.tile([C2, R], fp32, tag="ot")
            nc.vector.tensor_copy(OT[:], po[:])
            nc.sync.dma_start(out_t[:], OT[:])

    nc.compile()
    return nc


def _host_precompute(x, lower_lp, upper_lp, up_W, low_W, h_W):
    """PRNG reproduction + scaling; returns per-core input maps and G."""
    import jax
    import jax.numpy as jnp
    import ml_dtypes

    E4 = ml_dtypes.float8_e4m3
    ADT = (E4, E4, E4)
    cpu = jax.devices("cpu")[0]
    f32 = np.float32

    with jax.default_device(cpu):
        key = jax.random.key(1)
        keys = jax.random.split(key, NSHIFT)
        fads, gs = [], []
        for i in range(NSHIFT):
            kf, kn = jax.random.split(keys[i])
            kr, ki = jax.random.split(kf)
            re = jax.random.normal(kr, (N, N), jnp.float32) * CF_COMP_STD
            im = jax.random.normal(ki, (N, N), jnp.float32) * CF_COMP_STD
            fads.append(np.asarray(jnp.sqrt(re * re + im * im)))
            gs.append(np.asarray(jax.random.normal(kn, (N, C), jnp.float32)))

    # fp32 replica of the up-branch batch-0 chain -> noise stds and scales
    stds = []
    z = x[0].astype(f32)
    for i in range(NSHIFT):
        stds.append(f32(np.sqrt(np.mean(z * z) / SNR_LIN)))
        z = (upper_lp * fads[i]).astype(f32) @ z + stds[i] * gs[i]
    r_last = f32(np.sqrt(np.mean(z * z)))
    r = [f32(stds[i + 1] * np.sqrt(SNR_LIN)) for i in range(NSHIFT - 1)]
    r.append(r_last)
    r_in = f32(np.sqrt(np.mean(x[0].astype(f32) ** 2)))
    G = float(r[-1])

    # big shift matrices: (lp * fad).T * sa_k in per-shift fp8, column-sliced
    # per core and pre-tiled partition-major:
    #   a_k[br*128+p, j*512+m] = (AT*sa_k)[j*128+p, d*R+m]
    a_cores = [[np.empty((2 * 128, NJ * R), ADT[k]) for k in range(NSHIFT)]
               for _ in range(NCORES)]
    for k in range(NSHIFT):
        for br, lp in ((0, upper_lp), (1, lower_lp)):
            at8 = np.ascontiguousarray(
                (lp * fads[k]).T * f32(SA[k])).astype(ADT[k])
            row0 = br * 128
            for d in range(NCORES):
                blk = at8[:, d * R:(d + 1) * R]          # [N, R]
                a_cores[d][k][row0:row0 + 128, :] = (
                    blk.reshape(NJ, 128, R).transpose(1, 0, 2)
                       .reshape(128, NJ * R))

    # normalized input, both batches side by side: X[n, c2]
    X = np.empty((N, C2), f32)
    X[:, :C] = x[0].astype(f32) / r_in
    X[:, C:] = x[1].astype(f32) / r_in
    # e4m3 hi/lo pair; SBUF layout [p, j*(2*C2) + t*C2 + c2]
    Xhi = X.astype(E4)
    Xlo = (X - Xhi.astype(f32)).astype(E4)
    x0p = np.empty((N, 2, C2), E4)
    x0p[:, 0, :] = Xhi
    x0p[:, 1, :] = Xlo
    x0_sb = np.ascontiguousarray(
        x0p.reshape(NJ, 128, 2 * C2).transpose(1, 0, 2).reshape(128, -1))

    # per-core transposed input slice for the h_W projection
    X16 = X.astype(np.float16)
    xt0_cores = [np.ascontiguousarray(X16[d * R:(d + 1) * R, :].T)
                 for d in range(NCORES)]

    # per-core noise slices (x SX_k), transposed + duplicated for batches
    nz_cores = [np.empty((C2, NSHIFT * R), np.float16) for _ in range(NCORES)]
    for k in range(NSHIFT):
        nT = np.ascontiguousarray(
            (f32(SX[k]) * (stds[k] / r[k]) * gs[k]).astype(f32).T)
        for d in range(NCORES):
            sl = nT[:, d * R:(d + 1) * R].astype(np.float16)
            nz_cores[d][:C, k * R:(k + 1) * R] = sl
            nz_cores[d][C:, k * R:(k + 1) * R] = sl

    # projection weights: shift terms fold 1/SX_k (yt = SX_k * y_norm)
    wc_np = np.zeros((C2, NTERM * C2), np.float16)
    terms = []
    for k in range(NSHIFT):
        terms.append((f32(r[k] / (G * SX[k])), up_W[k]))
        terms.append((f32(r[k] / (G * SX[k])), low_W[k]))
    terms.append((f32(r_in / G), h_W))
    for ti, (scale, W) in enumerate(terms):
        blk = (scale * W.astype(f32)).T.astype(np.float16)  # [c, o]
        wc_np[:C, ti * C2:ti * C2 + C] = blk
        wc_np[C:, ti * C2 + C:(ti + 1) * C2] = blk

    # per-shift psum scales:
    #   bt_0 = SX_0 * beta_0 / SA_0            (x0 carried unscaled)
    #   bt_k = (SX_k / SX_{k-1}) * beta_k / SA_k
    bt_np = np.empty((128, NSHIFT), f32)
    bt_np[:, 0] = f32(SX[0] * (r_in / r[0]) / SA[0])
    for k in range(1, NSHIFT):
        bt_np[:, k] = f32((SX[k] / SX[k - 1]) * (r[k - 1] / r[k]) / SA[k])

    in_maps = []
    for d in range(NCORES):
        in_maps.append({
            "a0": a_cores[d][0],
            "a1": a_cores[d][1],
            "a2": a_cores[d][2],
            "x0": x0_sb,
            "xt0": xt0_cores[d],
            "nz": nz_cores[d],
            "wc": wc_np,
            "bt": bt_np,
            "idn": np.eye(128, dtype=np.float16),
        })
    return in_maps, G


def kernel(x, lower_lp, upper_lp, up_W, low_W, h_W):
    global LAST_RESULTS
    from concourse.bass_utils import run_bass_kernel_spmd

    x = np.asarray(x, np.float32)
    lower_lp = np.asarray(lower_lp, np.float32)
    upper_lp = np.asarray(upper_lp, np.float32)
    up_W = np.asarray(up_W, np.float32)
    low_W = np.asarray(low_W, np.float32)
    h_W = np.asarray(h_W, np.float32)

    in_maps, G = _host_precompute(x, lower_lp, upper_lp, up_W, low_W, h_W)

    if "nc" not in _compiled:
        _compiled["nc"] = _build_nc()
    nc = _compiled["nc"]

    trace = os.environ.get("AIRTNN_TRACE", "0") == "1"
    res = run_bass_kernel_spmd(nc, in_maps, list(range(NCORES)), trace=trace)
    LAST_RESULTS = res

    # out[b, d*R + m, o] = G * out_t_d[o + 64*b, m]
    out = np.empty((B, N, C), np.float32)
    for d in range(NCORES):
        ot = res.results[d]["out_t"]  # [C2, R] fp32
        for b in range(B):
            out[b, d * R:(d + 1) * R, :] = (ot[b * C:(b + 1) * C, :].T) * G
    return out
